# revision 27
# baseline (speedup 1.0000x reference)
"""Additive-attention kernel for 8 TRN2 NeuronCores — fp8 DoubleRow version.

reference:
    x = concat([s, h], axis=1)            # (N, 2D)
    X = tanh(x @ W.T)                     # (N, 2*DA)
    pre = (X @ v.T).T                     # (1, N)
    out = softmax(pre, axis=1)            # (1, N)

Strategy: shard rows (N) across 8 cores (4096 rows each). W, v replicated.
The z = x @ W.T contraction (2048 = 16 k-tiles of 128) is computed in mixed
precision: the first 12 k-tiles in fp8 e4m3 with perf_mode=DoubleRow (two
k-tiles per matmul, ~1.44x bf16 throughput), the last 4 k-tiles in bf16.
Both operand sets are pre-scaled on host (x*4, W*64 — powers of two, exact)
so all products share one PSUM scale; tanh applies scale=1/256.

fp8 quantization error is compensated to first order: with xres = x - xeff,
Wres = W - Weff (effective dequantized operands), the score error is
  dscore_i ~= sum_j v_j tanh'(z_ij) dz_ij ~= alpha * [xres_i.u + xeff_i.r]
with u = v@W, r = v@Wres, alpha = E[sech^2(z)].  u, r and the per-row dot
products are O(N*D) host matvecs; the resulting per-row correction c_i is
shipped as a tiny [P, MT] tensor and added to the scores before the softmax.
Measured (host-sim, matches hw to ~1e-6): rel err 3.1e-2 uncorrected,
1.44e-2 corrected at the 12/4 fp8/bf16 split (gate: 2e-2).

Per core, per row-tile (128 rows): 6 DoubleRow MMs + 4 bf16 MMs per psum
chunk x 4 chunks, k-pair-outer / chunk-inner so 4 consecutive MMs share the
stationary x operand (1 LDWEIGHTS per 4 MMs after stripping).  Scores ->
exp -> local sum -> AllGather(8) -> normalize, as in the bf16 baseline.
"""

import numpy as np
import ml_dtypes

N, D, DA = 32768, 1024, 1024
NCORES = 8
NS = N // NCORES            # 4096 rows per core
P = 128
MT = NS // P                # 32 row-tiles per core
KIN = 2 * D                 # 2048 contraction
KT = KIN // P               # 16 k-tiles
NOUT = 2 * DA               # 2048 out features
NCH = 512                   # psum chunk (one bank of fp32)
NCK = NOUT // NCH           # 4 chunks

KB = 0                      # k-tiles computed in bf16 (accuracy dial)
K8 = KT - KB                # k-tiles computed in fp8 DoubleRow (must be even)
NP8 = K8 // 2               # DoubleRow k-pairs
BX = 4.0                    # host pre-scale on x (power of 2)
BW = 64.0                   # host pre-scale on W (power of 2)
TANH_SCALE = 1.0 / (BX * BW)
# first-order-corrected error vs slope alpha is V-shaped with a flat
# basin; the analytic alpha=E[sech^2(z)]~0.585 sits above the basin floor.
# Host-sim sweep: KB=0 -> best 1.773e-2 at 0.49; KB=2 -> 1.618e-2 (sample
# alpha fine).  Override used when set:
ALPHA_OVERRIDE = {0: 0.49}.get(KB)


def _build_nc():
    from concourse import bacc, mybir, tile, bass

    f32 = mybir.dt.float32
    bf16 = mybir.dt.bfloat16
    fp8 = mybir.dt.float8e4
    AF = mybir.ActivationFunctionType
    ALU = mybir.AluOpType
    AX = mybir.AxisListType
    DR = mybir.MatmulPerfMode.DoubleRow

    nc = bacc.Bacc(
        "TRN2",
        target_bir_lowering=False,
        debug=False,
        num_devices=NCORES,
    )

    x8e = nc.declare_dram_parameter("x8", [NS, K8 * P], fp8, isOutput=False)
    w8e = nc.declare_dram_parameter("w8", [K8 * P, NOUT], fp8, isOutput=False)
    if KB:
        xbe = nc.declare_dram_parameter(
            "xb", [NS, KB * P], bf16, isOutput=False
        )
        wbe = nc.declare_dram_parameter(
            "wb", [KB * P, NOUT], bf16, isOutput=False
        )
    vr = nc.declare_dram_parameter("vr", [P, NOUT], f32, isOutput=False)
    cve = nc.declare_dram_parameter("cv", [P, MT], f32, isOutput=False)
    out_ext = nc.declare_dram_parameter("out", [P, MT], f32, isOutput=True)

    with tile.TileContext(nc) as tc:
        with (
            tc.tile_pool(name="wpool", bufs=1) as wpool,
            tc.tile_pool(name="xpool", bufs=5) as xpool,
            tc.tile_pool(name="tpool", bufs=3) as tpool,
            tc.tile_pool(name="spool", bufs=1) as spool,
            tc.tile_pool(name="ppool", bufs=2, space="PSUM") as ppool,
            tc.tile_pool(name="dpool", bufs=1, space="DRAM") as dpool,
        ):
            def load_xm(m, eng=None):
                t8 = xpool.tile([P, K8, P], fp8, name="xm8", tag="xm8")
                # gpsimd queue: the sync/scalar queues stream the weight
                # bulk for the first ~15us, and an x tile queued behind
                # them starves the PE (measured 3.7us gap at tile 3)
                e = eng or nc.gpsimd
                e.dma_start(out=t8[:, :, :], in_=x8e[m * P:(m + 1) * P, :])
                if KB:
                    tb = xpool.tile([P, KB, P], bf16, name="xmb", tag="xmb")
                    e.dma_start(
                        out=tb[:, :, :], in_=xbe[m * P:(m + 1) * P, :]
                    )
                else:
                    tb = None
                return t8, tb

            # first row-tile: the k0/k1 slice lands first so the first
            # DoubleRow matmul (which reads xm8[:, 0:2, :]) starts as soon
            # as w8's first pair arrives; issues spread across engine queues
            xm8_0 = xpool.tile([P, K8, P], fp8, name="xm8", tag="xm8")
            nc.sync.dma_start(out=xm8_0[:, 0:2, :], in_=x8e[0:P, 0:2 * P])

            # the NEFF entry barrier (bir_kernel_barrier, a CC op Bacc
            # inserts) already absorbs launch skew; to keep the CC cores
            # warm through the ~300us matmul phase we fire tiny dummy
            # AllGathers from inside the row-tile loop (see below), so the
            # real AllGather at the softmax doesn't pay a cold-CC penalty
            warm_ins = [
                dpool.tile([1, 1], f32, name=f"warm_in{i}") for i in range(3)
            ]
            warm_outs = [
                dpool.tile(
                    [1, NCORES], f32, name=f"warm_out{i}", addr_space="Shared"
                )
                for i in range(3)
            ]

            def cc_warm(i, dep_ap):
                # the dma from freshly-written scores data ties the warmer
                # to row-tile progress, so the gpsimd queue can't race ahead
                # and fire all warmers at t~0
                nc.gpsimd.dma_start(out=warm_ins[i][0:1, 0:1], in_=dep_ap)
                nc.gpsimd.collective_compute(
                    "AllGather",
                    ALU.bypass,
                    replica_groups=[list(range(NCORES))],
                    ins=[warm_ins[i].opt()],
                    outs=[warm_outs[i].opt()],
                )

            # fp8 weights: [128, K8, NOUT]; k-pair t is [:, 2t:2t+2, :].
            # The first DoubleRow matmul reads [:, 0:2, 0:512]: put exactly
            # those two slot-slices first, on otherwise-idle queues (vector/
            # gpsimd), so the first matmul's deps land ~1us after queue
            # start; the bulk streams behind on sync/scalar
            w8sb = wpool.tile([P, K8, NOUT], fp8, name="w8sb")
            nc.scalar.dma_start(out=w8sb[:, 0, 0:NCH], in_=w8e[0:P, 0:NCH])
            nc.gpsimd.dma_start(
                out=w8sb[:, 1, 0:NCH], in_=w8e[P:2 * P, 0:NCH]
            )
            nc.scalar.dma_start(
                out=w8sb[:, 0, NCH:NOUT], in_=w8e[0:P, NCH:NOUT]
            )
            nc.scalar.dma_start(
                out=w8sb[:, 1, NCH:NOUT], in_=w8e[P:2 * P, NCH:NOUT]
            )
            # remaining first-tile x slices and the k>=2 weight stream,
            # interleaved on alternating queues so pair t's operands land
            # just ahead of its matmuls
            nc.sync.dma_start(out=w8sb[:, 2, :], in_=w8e[2 * P:3 * P, :])
            nc.gpsimd.dma_start(out=w8sb[:, 3, :], in_=w8e[3 * P:4 * P, :])
            nc.sync.dma_start(
                out=xm8_0[:, 2:4, :], in_=x8e[0:P, 2 * P:4 * P]
            )
            nc.sync.dma_start(
                out=xm8_0[:, 4:K8, :], in_=x8e[0:P, 4 * P:K8 * P]
            )
            if KB:
                xmb_0 = xpool.tile([P, KB, P], bf16, name="xmb", tag="xmb")
                nc.gpsimd.dma_start(out=xmb_0[:, :, :], in_=xbe[0:P, :])
            else:
                xmb_0 = None
            xm_pre = [(xm8_0, xmb_0)] + [
                load_xm(m, nc.gpsimd) for m in (1, 2, 3)
            ]
            for k in range(4, K8):
                eng = nc.sync if k % 2 == 0 else nc.scalar
                eng.dma_start(
                    out=w8sb[:, k, :], in_=w8e[k * P:(k + 1) * P, :]
                )
            if KB:
                wbsb = wpool.tile([P, KB, NOUT], bf16, name="wbsb")
                for k in range(KB):
                    nc.sync.dma_start(
                        out=wbsb[:, k, :], in_=wbe[k * P:(k + 1) * P, :]
                    )
            vsb = wpool.tile([P, NOUT], f32, name="vsb")
            nc.sync.dma_start(out=vsb[:, :], in_=vr[:, :])
            csb = spool.tile([P, MT], f32, name="csb")
            nc.gpsimd.dma_start(out=csb[:, :], in_=cve[:, :])

            scores = spool.tile([P, MT], f32, name="scores")
            expv = spool.tile([P, MT], f32, name="expv")
            zrow = spool.tile([P, 1], f32, name="zrow")

            for m in range(MT):
                xm8, xmb = xm_pre[m] if m < len(xm_pre) else load_xm(m)
                if m in (8, 16, 24):
                    # depends on the previous row-tile's freshly-written score
                    cc_warm(m // 8 - 1, scores[0:1, m - 1:m])

                # one 4-bank psum tile; each 512-slice is bank-aligned
                pst = ppool.tile([P, NOUT], f32, name="pst", tag="pst")
                tmt = tpool.tile([P, NOUT], f32, name="tmt", tag="tmt")
                umt = tpool.tile([P, NOUT], f32, name="umt", tag="umt")

                # k-pair outer / chunk inner: 4 consecutive matmuls share the
                # stationary x operand (1 LDWEIGHTS per 4 MMs after
                # stripping); each psum bank's group opens at t=0 and closes
                # at the last bf16 k-tile
                acc = tpool.tile([P, NCK], f32, name="acc", tag="acc")
                for t in range(NP8):
                    for j in range(NCK):
                        nc.tensor.matmul(
                            pst[:, j * NCH:(j + 1) * NCH],
                            lhsT=xm8[:, 2 * t:2 * t + 2, :],
                            rhs=w8sb[:, 2 * t:2 * t + 2, j * NCH:(j + 1) * NCH],
                            start=(t == 0),
                            stop=(KB == 0 and t == NP8 - 1),
                            perf_mode=DR,
                        )
                for k in range(KB):
                    for j in range(NCK):
                        nc.tensor.matmul(
                            pst[:, j * NCH:(j + 1) * NCH],
                            lhsT=xmb[:, k, :],
                            rhs=wbsb[:, k, j * NCH:(j + 1) * NCH],
                            start=False,
                            stop=(k == KB - 1),
                        )

                # per-chunk tanh+reduce: 4 smaller ACT/DVE op pairs pipeline
                # against each other, shortening the post-matmul latency on
                # the last row-tile vs one [P, NOUT] op pair
                for j in range(NCK):
                    sl = slice(j * NCH, (j + 1) * NCH)
                    nc.scalar.activation(
                        tmt[:, sl], pst[:, sl], AF.Tanh, scale=TANH_SCALE
                    )
                    # one DVE op: umt = tanh*v, acc[:,j] = row-sum(umt)
                    nc.vector.scalar_tensor_tensor(
                        out=umt[:, sl],
                        in0=tmt[:, sl],
                        scalar=1.0,
                        in1=vsb[:, sl],
                        op0=ALU.mult,
                        op1=ALU.mult,
                        accum_out=acc[:, j:j + 1],
                    )
                nc.vector.tensor_reduce(
                    scores[:, m:m + 1], acc[:, :], AX.X, ALU.add
                )

            # add the host-computed fp8 first-order correction, then
            # softmax over the global N via one AllGather
            nc.vector.scalar_tensor_tensor(
                out=scores[:, :],
                in0=scores[:, :],
                scalar=1.0,
                in1=csb[:, :],
                op0=ALU.mult,
                op1=ALU.add,
            )
            nc.scalar.activation(expv[:, :], scores[:, :], AF.Exp)
            nc.vector.tensor_reduce(zrow[:, 0:1], expv[:, :], AX.X, ALU.add)
            zloc = spool.tile([1, 1], f32, name="zloc")
            nc.gpsimd.tensor_reduce(
                zloc[0:1, 0:1], zrow[:, 0:1], AX.C, ALU.add
            )
            zin = dpool.tile([1, 1], f32, name="zin")
            zout = dpool.tile([1, NCORES], f32, name="zout", addr_space="Shared")
            nc.sync.dma_start(out=zin[:, :], in_=zloc[0:1, 0:1])
            nc.gpsimd.collective_compute(
                "AllGather",
                ALU.bypass,
                replica_groups=[list(range(NCORES))],
                ins=[zin.opt()],
                outs=[zout.opt()],
            )
            # DMA the gathered 8 partials to every partition (stride-0 DRAM
            # read), reduce and reciprocal per partition, then scale
            zgb = spool.tile([P, NCORES], f32, name="zgb")
            zout_bc = bass.AP(
                zout.tensor, zout.offset, [(0, P), (1, NCORES)]
            )
            nc.sync.dma_start(out=zgb[:, :], in_=zout_bc)
            zp = spool.tile([P, 1], f32, name="zp")
            nc.vector.tensor_reduce(zp[:, 0:1], zgb[:, :], AX.X, ALU.add)
            rzb = spool.tile([P, 1], f32, name="rzb")
            nc.vector.reciprocal(rzb[:, 0:1], zp[:, 0:1])
            outsb = spool.tile([P, MT], f32, name="outsb")
            nc.vector.tensor_scalar_mul(outsb[:, :], expv[:, :], rzb[:, 0:1])
            nc.sync.dma_start(out=out_ext[:, :], in_=outsb[:, :])

    # run_bass_via_pjrt binds the exec primitive directly and skips the
    # finalize that bass_jit flows do; Bacc register allocation runs here.
    nc.finalize()
    _strip_redundant_ldweights(nc)
    return nc


def _strip_redundant_ldweights(nc):
    """Bacc's move_matmul_waits_to_ldweights emits one InstLdweights per
    matmul even when consecutive matmuls share the stationary operand.
    The PE keeps the loaded weights across matmuls, so an Ldweights whose
    weights AP equals the previous one's and that carries no semaphore
    waits/updates is pure redundant load time (~110ns each on the PE
    critical path). Drop them; only the matmuls (ldweights=false) remain."""
    def sig(arg):
        return (
            getattr(arg, "memref", None),
            getattr(arg, "offset", None),
            str(getattr(arg, "ap", None)),
        )

    removed = 0
    for bb in nc.main_func.blocks:
        keep = []
        last = None
        for inst in bb.instructions:
            if "Ldweights" in type(inst).__name__:
                s = sig(inst.ins[0])
                si = inst.sync_info
                if s == last and (
                    si is None or (not si.on_wait and not si.on_update)
                ):
                    removed += 1
                    continue
                last = s
            keep.append(inst)
        bb.instructions = keep
    return removed


def _quantize(s, h, W):
    """Quantize operands the way the device consumes them and compute the
    first-order score correction.  Returns per-core input arrays."""
    e4 = ml_dtypes.float8_e4m3
    bf = ml_dtypes.bfloat16
    K8C = K8 * P                                          # fp8 columns

    x = np.concatenate([s, h], axis=1)                    # [N, KIN] f32
    q8 = (x[:, :K8C] * BX).astype(e4)                     # [N, K8C]
    qb = (x[:, K8C:] * BX).astype(bf)                     # [N, KB*P]
    W8 = (W[:, :K8C].T * BW).astype(e4)                   # [K8C, NOUT] (W.T)
    Wb = (W[:, K8C:].T * BW).astype(bf)                   # [KB*P, NOUT]
    return x, q8, qb, W8, Wb


def _sample_alpha(xeff, Weff):
    zs = xeff[:256] @ Weff.T
    return float(np.mean(1.0 - np.tanh(zs) ** 2))


def _correction(x, q8, qb, W, v, W8, Wb):
    """Per-row first-order correction c_i = alpha*(xres_i.u + xeff_i.r)."""
    K8C = K8 * P
    xeff = np.empty_like(x)
    xeff[:, :K8C] = q8.astype(np.float32) / BX
    xeff[:, K8C:] = qb.astype(np.float32) / BX
    Weff = np.empty_like(W)                               # [NOUT, KIN]
    Weff[:, :K8C] = W8.astype(np.float32).T / BW
    Weff[:, K8C:] = Wb.astype(np.float32).T / BW

    v64 = v[0].astype(np.float64)
    u = (v64 @ W.astype(np.float64)).astype(np.float32)   # [KIN]
    r = (v64 @ (W.astype(np.float64) - Weff.astype(np.float64))).astype(
        np.float32
    )
    d = (x - xeff) @ u + xeff @ r                         # [N]

    alpha = (ALPHA_OVERRIDE if ALPHA_OVERRIDE is not None
             else _sample_alpha(xeff, Weff))
    return (alpha * d).astype(np.float32)


def _tile_transpose(a, kt):
    """[NS, kt*P] row-major -> device layout with each [P, P] block
    transposed: out[m*P+kk, k*P+rr] = a[m*P+rr, k*P+kk]."""
    return np.ascontiguousarray(
        a.reshape(MT, P, kt, P).transpose(0, 3, 2, 1).reshape(NS, kt * P)
    )


def _prep_core_inputs(s, h, W, v):
    x, q8, qb, W8, Wb = _quantize(s, h, W)
    c = _correction(x, q8, qb, W, v, W8, Wb)

    vrep = np.ascontiguousarray(
        np.broadcast_to(v.reshape(1, NOUT), (P, NOUT))
    ).astype(np.float32)
    W8 = np.ascontiguousarray(W8)
    Wb = np.ascontiguousarray(Wb)

    in_maps = []
    for cidx in range(NCORES):
        sl = slice(cidx * NS, (cidx + 1) * NS)
        m = {
            "x8": _tile_transpose(q8[sl], K8),
            "w8": W8,
            "vr": vrep,
            "cv": np.ascontiguousarray(c[sl].reshape(MT, P).T),
        }
        if KB:
            m["xb"] = _tile_transpose(qb[sl], KB)
            m["wb"] = Wb
        in_maps.append(m)
    return in_maps


_RUN_KW = {}  # test.py can inject trace=True etc.
LAST_RESULT = None


def kernel(s, h, W, v):
    from concourse.bass_utils import run_bass_kernel_spmd

    global LAST_RESULT
    s = np.asarray(s, dtype=np.float32)
    h = np.asarray(h, dtype=np.float32)
    W = np.asarray(W, dtype=np.float32)
    v = np.asarray(v, dtype=np.float32)

    in_maps = _prep_core_inputs(s, h, W, v)
    res = None
    for attempt in range(3):
        nc = _build_nc()
        try:
            res = run_bass_kernel_spmd(
                nc, in_maps, core_ids=list(range(NCORES)), **_RUN_KW
            )
            break
        except Exception:
            # transient NRT_EXEC_UNIT_UNRECOVERABLE states clear on the
            # next attempt; rebuild and retry
            if attempt == 2:
                raise
            import time
            time.sleep(15)
    LAST_RESULT = res

    outs = []
    for c in range(NCORES):
        oc = np.asarray(res.results[c]["out"], dtype=np.float32)  # [P, MT]
        outs.append(oc.T.reshape(-1))                              # rows m*128+p
    return np.concatenate(outs).reshape(1, N).astype(np.float32)
